# revision 1
# baseline (speedup 1.0000x reference)
"""KanLinear Trainium2 Bass kernel.

Math (reference):
    base_out  = silu(x) @ scale_base                     # [B,O]
    bases     = exp(-((x[:,:,None]-grid[None])/sigma)^2) # [B,I,G]
    spline    = einsum('big,oig,io->bo', bases, spline_weight, scale_spline)
    out       = base_out + spline

Strategy:
  - Data-parallel shard B=8192 across 8 cores (1024 rows each); params replicated.
  - Host does layout-only prep: x -> x^T slices [I, Bs]; spline_weight [O,I,G]
    -> wT [(g-major k)=G*I, O]; sigma broadcast to [128,1].
  - On device, everything lives in [i/k partitions, b free] layout:
      silu_t = Silu(x^T)                        (ACT, bf16 out)
      u      = Square(x^T * (1/sigma) - grid/sigma)   (ACT, per-partition bias/scale)
      bases  = Exp(-u)                          (ACT, bf16 out)
      w2     = wT_chunk * scale_spline_chunk    (DVE, bf16 out)
      psum[b,o] += silu^T@scale_base + bases^T@w2  (PE, fp32 accum)
  - Single [B, I*G+I] x [I*G+I, O] contraction accumulated in PSUM: for each
    of 4 o-blocks (256 cols), 8 PSUM banks hold the 8 b-blocks, k streams
    over 8 base chunks + 64 spline chunks.
"""

import time

import numpy as np
import orjson

import concourse.bass as bass
import concourse.mybir as mybir
import concourse.tile as tile

F32 = mybir.dt.float32
BF16 = mybir.dt.bfloat16

N_CORES = 8
B, I, O, G = 8192, 1024, 1024, 8
BS = B // N_CORES  # 1024 batch rows per core
P = 128            # partitions
IB = I // P        # 8 i-blocks
NB = BS // P       # 8 b-blocks
NO = 256           # o-block width (one PSUM bank tile = [128, 256] f32)
OB = O // NO       # 4 o-blocks
KC = G * IB        # 64 spline k-chunks of 128


MAX_WAIT_SLOTS = 1


def split_sync_waits(bir_json: bytes, max_waits: int = MAX_WAIT_SLOTS) -> bytes:
    """The walrus build in this container rejects instructions with more than
    `max_waits` semaphore wait slots ('Too many sync wait commands').  Move
    excess waits onto NoOps inserted just before the instruction on the same
    engine — the sequencer executes them in order, so the dependency semantics
    are identical."""
    m = orjson.loads(bir_json)
    n_new = 0
    for fn in m["functions"]:
        for blk in fn["blocks"]:
            out_insts = []
            changed = False
            for ins in blk["instructions"]:
                si = ins.get("sync_info")
                waits = (si or {}).get("on_wait") or []
                if len(waits) > max_waits:
                    chunks = [
                        waits[i : i + max_waits]
                        for i in range(0, len(waits), max_waits)
                    ]
                    for chunk in chunks[:-1]:
                        n_new += 1
                        out_insts.append(
                            {
                                "name": f"I-WSPLIT{n_new}",
                                "opcode": "NoOp",
                                "engine": ins["engine"],
                                "ins": [],
                                "outs": [],
                                "sync_info": {"on_wait": chunk, "on_update": []},
                            }
                        )
                    si["on_wait"] = chunks[-1]
                    changed = True
                out_insts.append(ins)
            if changed:
                blk["instructions"] = out_insts
    return orjson.dumps(m)


def install_wait_split_hook():
    """Route every compile through split_sync_waits."""
    from concourse import bass2jax

    if getattr(bass2jax.compile_bir_kernel, "_wait_split", False):
        return
    orig = bass2jax.compile_bir_kernel

    def patched(bir_json, tmpdir, neff_name="file.neff"):
        return orig(split_sync_waits(bir_json), tmpdir, neff_name)

    patched._wait_split = True
    bass2jax.compile_bir_kernel = patched


def build_bass():
    nc = bass.Bass("TRN2", target_bir_lowering=False, debug=False, num_devices=N_CORES)

    xT = nc.dram_tensor("xT", [I, BS], F32, kind="ExternalInput").ap()
    wT = nc.dram_tensor("wT", [G * I, O], F32, kind="ExternalInput").ap()
    sb = nc.dram_tensor("sb", [I, O], F32, kind="ExternalInput").ap()
    ss = nc.dram_tensor("ss", [I, O], F32, kind="ExternalInput").ap()
    grid = nc.dram_tensor("grid", [I, G], F32, kind="ExternalInput").ap()
    sigma = nc.dram_tensor("sigma", [P, 1], F32, kind="ExternalInput").ap()
    out = nc.dram_tensor("out", [BS, O], F32, kind="ExternalOutput").ap()

    AF = mybir.ActivationFunctionType
    ALU = mybir.AluOpType

    with tile.TileContext(nc) as tc:
        with (
            tc.tile_pool(name="const", bufs=1) as const_pool,
            tc.tile_pool(name="xp", bufs=2) as x_pool,
            tc.tile_pool(name="s2p", bufs=2) as s2_pool,
            tc.tile_pool(name="vp", bufs=4) as v_pool,
            tc.tile_pool(name="silu", bufs=1) as silu_pool,
            tc.tile_pool(name="bases", bufs=1) as bases_pool,
            tc.tile_pool(name="ssp", bufs=2) as ss_pool,
            tc.tile_pool(name="wp", bufs=4) as w_pool,
            tc.tile_pool(name="w2p", bufs=4) as w2_pool,
            tc.tile_pool(name="sbf", bufs=2) as sbf_pool,
            tc.tile_pool(name="sbb", bufs=2) as sbb_pool,
            tc.tile_pool(name="psum", bufs=1, space="PSUM") as psum_pool,
            tc.tile_pool(name="op", bufs=3) as out_pool,
        ):
            # ---- phase 0: constants -------------------------------------
            sig_t = const_pool.tile([P, 1], F32, tag="sig")
            nc.sync.dma_start(sig_t[:], sigma[:])
            inv_t = const_pool.tile([P, 1], F32, tag="inv")
            nc.vector.reciprocal(inv_t[:], sig_t[:])

            # RBF via expansion: -(x/s - g/s)^2 = -(x/s)^2 + (2g/s^2)x - (g/s)^2
            #   s2   = Square(x * (1/s))                  (ACT, per ib)
            #   v_g  = (x * c1_g) - s2                    (DVE/GpSimd stt)
            #   base = Exp(v_g + c2_g)                    (ACT)
            # with c1_g = 2*grid/s^2, c2_g = -(grid/s)^2 as [128,1] scalars.
            c1_t, c2_t = [], []
            for ib in range(IB):
                g_t = const_pool.tile([P, G], F32, tag=f"grid{ib}")
                nc.sync.dma_start(g_t[:], grid[ib * P : (ib + 1) * P, :])
                gs_t = const_pool.tile([P, G], F32, tag=f"gs{ib}")
                nc.vector.tensor_scalar_mul(gs_t[:], g_t[:], inv_t[:, 0:1])
                c1 = const_pool.tile([P, G], F32, tag=f"c1_{ib}")
                nc.vector.tensor_scalar(
                    c1[:], gs_t[:], inv_t[:, 0:1], 2.0, ALU.mult, ALU.mult
                )
                c2 = const_pool.tile([P, G], F32, tag=f"c2_{ib}")
                nc.vector.scalar_tensor_tensor(
                    c2[:], gs_t[:], -1.0, gs_t[:], ALU.mult, ALU.mult
                )
                c1_t.append(c1)
                c2_t.append(c2)

            # ---- phase 1: silu + RBF bases ------------------------------
            silu_t = [None] * IB
            bases_t = [None] * (G * IB)
            for ib in range(IB):
                x_t = x_pool.tile([P, BS], F32)
                nc.sync.dma_start(x_t[:], xT[ib * P : (ib + 1) * P, :])
                s_t = silu_pool.tile([P, BS], BF16, tag=f"silu{ib}")
                nc.scalar.activation(s_t[:], x_t[:], AF.Silu)
                silu_t[ib] = s_t
                s2_t = s2_pool.tile([P, BS], F32)
                nc.scalar.activation(s2_t[:], x_t[:], AF.Square, scale=inv_t[:, 0:1])
                for g in range(G):
                    kc = ib * G + g
                    v_t = v_pool.tile([P, BS], F32)
                    # (walrus here rejects TensorScalarPtr on GpSimd/Pool)
                    eng = nc.vector
                    eng.scalar_tensor_tensor(
                        v_t[:],
                        x_t[:],
                        c1_t[ib][:, g : g + 1],
                        s2_t[:],
                        ALU.mult,
                        ALU.subtract,
                    )
                    bt = bases_pool.tile([P, BS], BF16, tag=f"bases{kc}")
                    nc.scalar.activation(
                        bt[:], v_t[:], AF.Exp, bias=c2_t[ib][:, g : g + 1]
                    )
                    bases_t[kc] = bt

            # ---- phase 2: matmuls ---------------------------------------
            # k-chunk order kc = ib*G + g matches phase-1 production order,
            # so the PE never waits long for a base tile.
            for ob in range(OB):
                o0 = ob * NO
                psums = [
                    psum_pool.tile([P, NO], F32, tag=f"ps{b}", name=f"ps{ob}_{b}")
                    for b in range(NB)
                ]

                for kc in range(KC):
                    ib, g = kc // G, kc % G
                    if g == 0:
                        # base-matmul chunk + fresh scale_spline chunk per ib
                        sb_f = sbf_pool.tile([P, NO], F32)
                        nc.sync.dma_start(
                            sb_f[:], sb[ib * P : (ib + 1) * P, o0 : o0 + NO]
                        )
                        sb_b = sbb_pool.tile([P, NO], BF16)
                        nc.vector.tensor_copy(sb_b[:], sb_f[:])
                        for b in range(NB):
                            nc.tensor.matmul(
                                psums[b][:],
                                silu_t[ib][:, b * P : (b + 1) * P],
                                sb_b[:],
                                start=(ib == 0),
                                stop=False,
                            )
                        ss_c = ss_pool.tile([P, NO], F32)
                        nc.sync.dma_start(
                            ss_c[:], ss[ib * P : (ib + 1) * P, o0 : o0 + NO]
                        )
                    w_t = w_pool.tile([P, NO], F32)
                    nc.sync.dma_start(w_t[:], wT[kc * P : (kc + 1) * P, o0 : o0 + NO])
                    w2_t = w2_pool.tile([P, NO], BF16)
                    nc.vector.tensor_mul(w2_t[:], w_t[:], ss_c[:])
                    last = kc == KC - 1
                    for b in range(NB):
                        nc.tensor.matmul(
                            psums[b][:],
                            bases_t[kc][:, b * P : (b + 1) * P],
                            w2_t[:],
                            start=False,
                            stop=last,
                        )

                # drain PSUM -> SBUF -> DRAM
                for b in range(NB):
                    o_t = out_pool.tile([P, NO], F32)
                    nc.vector.tensor_copy(o_t[:], psums[b][:])
                    nc.sync.dma_start(
                        out[b * P : (b + 1) * P, o0 : o0 + NO], o_t[:]
                    )

    return nc


# ---------------------------------------------------------------------------
# host-side runner: build + compile once, then execute on 8 cores via PJRT
# ---------------------------------------------------------------------------
_STATE = {}


def _get_runner():
    if "run" in _STATE:
        return _STATE["run"]

    import jax
    from jax.sharding import Mesh, PartitionSpec
    from jax.experimental.shard_map import shard_map
    from concourse import bass2jax
    from concourse import mybir as _mb

    nc = build_bass()
    install_wait_split_hook()
    bass2jax.install_neuronx_cc_hook()

    partition_name = nc.partition_id_tensor.name if nc.partition_id_tensor else None
    in_names, out_names, out_avals, zero_shapes = [], [], [], []
    for alloc in nc.m.functions[0].allocations:
        if not isinstance(alloc, _mb.MemoryLocationSet):
            continue
        name = alloc.memorylocations[0].name
        if alloc.kind == "ExternalInput":
            if name != partition_name:
                in_names.append(name)
        elif alloc.kind == "ExternalOutput":
            out_names.append(name)
            shape = tuple(alloc.tensor_shape)
            dtype = _mb.dt.np(alloc.dtype)
            out_avals.append(jax.core.ShapedArray(shape, dtype))
            zero_shapes.append((shape, dtype))
    n_params = len(in_names)
    n_outs = len(out_avals)
    all_in_names = in_names + out_names
    if partition_name is not None:
        all_in_names = all_in_names + [partition_name]

    donate = tuple(range(n_params, n_params + n_outs))

    def _body(*args):
        operands = list(args)
        if partition_name is not None:
            operands.append(bass2jax.partition_id_tensor())
        outs = bass2jax._bass_exec_p.bind(
            *operands,
            out_avals=tuple(out_avals),
            in_names=tuple(all_in_names),
            out_names=tuple(out_names),
            lowering_input_output_aliases=(),
            sim_require_finite=True,
            sim_require_nnan=True,
            nc=nc,
        )
        return tuple(outs)

    devices = jax.devices()[:N_CORES]
    mesh = Mesh(np.asarray(devices), ("core",))
    specs = (PartitionSpec("core"),) * (n_params + n_outs)
    sharded = jax.jit(
        shard_map(
            _body,
            mesh=mesh,
            in_specs=specs,
            out_specs=(PartitionSpec("core"),) * n_outs,
            check_rep=False,
        ),
        donate_argnums=donate,
        keep_unused=True,
    )

    def run(in_maps):
        concat_in = [
            np.concatenate([np.asarray(in_maps[c][nm]) for c in range(N_CORES)], axis=0)
            for nm in in_names
        ]
        concat_zeros = [
            np.zeros((N_CORES * s[0], *s[1:]), d) for (s, d) in zero_shapes
        ]
        out_arrs = sharded(*concat_in, *concat_zeros)
        return [
            {
                nm: np.asarray(out_arrs[i]).reshape(N_CORES, *out_avals[i].shape)[c]
                for i, nm in enumerate(out_names)
            }
            for c in range(N_CORES)
        ]

    from jax.sharding import NamedSharding

    sh = NamedSharding(mesh, PartitionSpec("core"))

    def prep(in_maps):
        concat_in = [
            np.concatenate([np.asarray(in_maps[c][nm]) for c in range(N_CORES)], axis=0)
            for nm in in_names
        ]
        dev_in = [jax.device_put(a, sh) for a in concat_in]
        jax.block_until_ready(dev_in)
        return dev_in

    def exec_once(dev_in):
        zeros = [
            jax.device_put(np.zeros((N_CORES * s[0], *s[1:]), d), sh)
            for (s, d) in zero_shapes
        ]
        jax.block_until_ready(zeros)
        t0 = time.perf_counter()
        outs = sharded(*dev_in, *zeros)
        jax.block_until_ready(outs)
        return time.perf_counter() - t0

    def timed(in_maps, iters=20):
        """Steady-state timing: inputs device-resident; only fresh donated
        zero output buffers are re-staged (outside the timed region)."""
        dev_in = prep(in_maps)
        times = [exec_once(dev_in) for _ in range(iters)]
        return min(times) * 1e9, times

    _STATE["run"] = run
    _STATE["timed"] = timed
    _STATE["prep"] = prep
    _STATE["exec"] = exec_once
    _STATE["nc"] = nc
    return run


def _make_in_maps(x, scale_base, spline_weight, scale_spline, grid, sigma):
    x = np.asarray(x, np.float32)
    scale_base = np.ascontiguousarray(np.asarray(scale_base, np.float32))
    scale_spline = np.ascontiguousarray(np.asarray(scale_spline, np.float32))
    grid = np.ascontiguousarray(np.asarray(grid, np.float32))
    sigma_b = np.full((P, 1), np.float32(np.asarray(sigma)), np.float32)

    xT = np.ascontiguousarray(x.T)  # [I, B]
    # k-chunk order kc = ib*G + g: rows [kc*128,(kc+1)*128) hold (i in ib-block,
    # fixed g) — matches the bases production order in phase 1.
    wT = np.ascontiguousarray(
        np.asarray(spline_weight, np.float32)
        .transpose(1, 2, 0)          # [I, G, O]
        .reshape(I // P, P, G, O)
        .transpose(0, 2, 1, 3)       # [IB, G, P, O]
        .reshape(G * I, O)
    )

    in_maps = []
    for c in range(N_CORES):
        in_maps.append(
            {
                "xT": np.ascontiguousarray(xT[:, c * BS : (c + 1) * BS]),
                "wT": wT,
                "sb": scale_base,
                "ss": scale_spline,
                "grid": grid,
                "sigma": sigma_b,
            }
        )
    return in_maps


def kernel(x, scale_base, spline_weight, scale_spline, grid, sigma):
    run = _get_runner()
    in_maps = _make_in_maps(x, scale_base, spline_weight, scale_spline, grid, sigma)
    results = run(in_maps)
    return np.concatenate([results[c]["out"] for c in range(N_CORES)], axis=0)


def timed_run(inputs, iters=20):
    """Min wall-clock (ns) of a steady-state device-resident invocation."""
    _get_runner()
    in_maps = _make_in_maps(**inputs)
    best_ns, times = _STATE["timed"](in_maps, iters)
    ms = ", ".join(f"{t * 1e3:.2f}" for t in sorted(times)[:5])
    print(f"  fastest runs (ms): {ms}")
    return best_ns


def profile_run(inputs, outdir):
    """Capture an NTFF profile of one execution (core 0) via the axon
    sidechannel; returns (exec_time_ns, perfetto_trace_path)."""
    import glob
    import os

    from trn_agent_boot.trn_boot import _ntff_profile_via_ctypes

    import gauge.profiler
    from concourse.bass_utils import FishPath

    _get_runner()
    in_maps = _make_in_maps(**inputs)
    dev_in = _STATE["prep"](in_maps)
    _STATE["exec"](dev_in)  # warmup

    os.makedirs(outdir, exist_ok=True)
    hook = _ntff_profile_via_ctypes("/opt/axon/libaxon_pjrt.so")
    with hook(outdir, [0]):
        _STATE["exec"](dev_in)

    ntffs = glob.glob(os.path.join(outdir, "*_body*.ntff")) or glob.glob(
        os.path.join(outdir, "*.ntff")
    )
    if not ntffs:
        raise RuntimeError(f"no NTFF files written to {outdir}")
    profile = gauge.profiler.Profile(
        profile_path=FishPath(outdir),
        kernel_dev_mode=True,
        profile_on_exit=False,
        bass_kernel=_STATE["nc"].m,
        offline_processing=True,
        fname="*_body*",
    )
    results = profile.to_perfetto(model_index=(0,))
    r = results[0]
    return r.exec_time_ns, r.trace_path



# revision 3
# speedup vs baseline: 1.8387x; 1.8387x over previous
"""KanLinear Trainium2 Bass kernel.

Math (reference):
    base_out  = silu(x) @ scale_base                     # [B,O]
    bases     = exp(-((x[:,:,None]-grid[None])/sigma)^2) # [B,I,G]
    spline    = einsum('big,oig,io->bo', bases, spline_weight, scale_spline)
    out       = base_out + spline

Strategy:
  - Data-parallel shard B=8192 across 8 cores (1024 rows each); params replicated.
  - Host does layout-only prep: x -> x^T slices [I, Bs]; spline_weight [O,I,G]
    -> wT [(g-major k)=G*I, O]; sigma broadcast to [128,1].
  - On device, everything lives in [i/k partitions, b free] layout:
      silu_t = Silu(x^T)                        (ACT, bf16 out)
      u      = Square(x^T * (1/sigma) - grid/sigma)   (ACT, per-partition bias/scale)
      bases  = Exp(-u)                          (ACT, bf16 out)
      w2     = wT_chunk * scale_spline_chunk    (DVE, bf16 out)
      psum[b,o] += silu^T@scale_base + bases^T@w2  (PE, fp32 accum)
  - Single [B, I*G+I] x [I*G+I, O] contraction accumulated in PSUM: for each
    of 4 o-blocks (256 cols), 8 PSUM banks hold the 8 b-blocks, k streams
    over 8 base chunks + 64 spline chunks.
"""

import time

import numpy as np
import orjson

import concourse.bass as bass
import concourse.mybir as mybir
import concourse.tile as tile

F32 = mybir.dt.float32
BF16 = mybir.dt.bfloat16

N_CORES = 8
B, I, O, G = 8192, 1024, 1024, 8
BS = B // N_CORES  # 1024 batch rows per core
P = 128            # partitions
IB = I // P        # 8 i-blocks
NB = BS // P       # 8 b-blocks
NO = 512           # o-block width (one full PSUM bank tile = [128, 512] f32)
OB = O // NO       # 2 o-blocks
KC = G * IB        # 64 spline k-chunks of 128


MAX_WAIT_SLOTS = 1


def split_sync_waits(bir_json: bytes, max_waits: int = MAX_WAIT_SLOTS) -> bytes:
    """The walrus build in this container rejects instructions with more than
    `max_waits` semaphore wait slots ('Too many sync wait commands').  Move
    excess waits onto NoOps inserted just before the instruction on the same
    engine — the sequencer executes them in order, so the dependency semantics
    are identical."""
    m = orjson.loads(bir_json)
    n_new = 0
    for fn in m["functions"]:
        for blk in fn["blocks"]:
            out_insts = []
            changed = False
            for ins in blk["instructions"]:
                si = ins.get("sync_info")
                waits = (si or {}).get("on_wait") or []
                if len(waits) > max_waits:
                    chunks = [
                        waits[i : i + max_waits]
                        for i in range(0, len(waits), max_waits)
                    ]
                    for chunk in chunks[:-1]:
                        n_new += 1
                        out_insts.append(
                            {
                                "name": f"I-WSPLIT{n_new}",
                                "opcode": "NoOp",
                                "engine": ins["engine"],
                                "ins": [],
                                "outs": [],
                                "sync_info": {"on_wait": chunk, "on_update": []},
                            }
                        )
                    si["on_wait"] = chunks[-1]
                    changed = True
                out_insts.append(ins)
            if changed:
                blk["instructions"] = out_insts
    return orjson.dumps(m)


def install_wait_split_hook():
    """Route every compile through split_sync_waits."""
    from concourse import bass2jax

    if getattr(bass2jax.compile_bir_kernel, "_wait_split", False):
        return
    orig = bass2jax.compile_bir_kernel

    def patched(bir_json, tmpdir, neff_name="file.neff"):
        return orig(split_sync_waits(bir_json), tmpdir, neff_name)

    patched._wait_split = True
    bass2jax.compile_bir_kernel = patched


def build_bass():
    nc = bass.Bass("TRN2", target_bir_lowering=False, debug=False, num_devices=N_CORES)

    xT = nc.dram_tensor("xT", [I, BS], F32, kind="ExternalInput").ap()
    wT = nc.dram_tensor("wT", [G * I, O], F32, kind="ExternalInput").ap()
    sb = nc.dram_tensor("sb", [I, O], F32, kind="ExternalInput").ap()
    ss = nc.dram_tensor("ss", [I, O], F32, kind="ExternalInput").ap()
    grid = nc.dram_tensor("grid", [I, G], F32, kind="ExternalInput").ap()
    sigma = nc.dram_tensor("sigma", [P, 1], F32, kind="ExternalInput").ap()
    out = nc.dram_tensor("out", [BS, O], F32, kind="ExternalOutput").ap()

    AF = mybir.ActivationFunctionType
    ALU = mybir.AluOpType

    with tile.TileContext(nc) as tc:
        with (
            tc.tile_pool(name="const", bufs=1) as const_pool,
            tc.tile_pool(name="xp", bufs=2) as x_pool,
            tc.tile_pool(name="s2p", bufs=2) as s2_pool,
            tc.tile_pool(name="vp", bufs=2) as v_pool,
            tc.tile_pool(name="rp", bufs=2) as r_pool,
            tc.tile_pool(name="silu", bufs=1) as silu_pool,
            tc.tile_pool(name="bases", bufs=1) as bases_pool,
            tc.tile_pool(name="ssp", bufs=2) as ss_pool,
            tc.tile_pool(name="wp", bufs=4) as w_pool,
            tc.tile_pool(name="w2p", bufs=4) as w2_pool,
            tc.tile_pool(name="sbf", bufs=2) as sbf_pool,
            tc.tile_pool(name="sbb", bufs=2) as sbb_pool,
            tc.tile_pool(name="psum", bufs=1, space="PSUM") as psum_pool,
            tc.tile_pool(name="op", bufs=3) as out_pool,
        ):
            # ---- phase 0: constants -------------------------------------
            sig_t = const_pool.tile([P, 1], F32, tag="sig")
            nc.sync.dma_start(sig_t[:], sigma[:])
            inv_t = const_pool.tile([P, 1], F32, tag="inv")
            nc.vector.reciprocal(inv_t[:], sig_t[:])

            # RBF bases via a per-g multiplicative recurrence (grid columns
            # are uniformly spaced within each row):
            #   b_0     = Exp((2g_0/s^2) x - (x/s)^2 - (g_0/s)^2)
            #   b_{k+1} = b_k * R * cc_k,  R = Exp((2h/s^2) x),
            #   cc_k    = Exp(-h (g_k + g_{k+1}) / s^2)        ([128,1] consts)
            # One DVE STT per grid step; ACT only does Silu/Square/Exp(R)/Exp(b0).
            c1_t, c2_t, cc_t, rsc_t = [], [], [], []
            for ib in range(IB):
                g_t = const_pool.tile([P, G], F32, tag=f"grid{ib}")
                nc.sync.dma_start(g_t[:], grid[ib * P : (ib + 1) * P, :])
                gs_t = const_pool.tile([P, G], F32, tag=f"gs{ib}")
                nc.vector.tensor_scalar_mul(gs_t[:], g_t[:], inv_t[:, 0:1])
                c1 = const_pool.tile([P, 1], F32, tag=f"c1_{ib}")
                nc.vector.tensor_scalar(
                    c1[:], gs_t[:, 0:1], inv_t[:, 0:1], 2.0, ALU.mult, ALU.mult
                )
                c2 = const_pool.tile([P, 1], F32, tag=f"c2_{ib}")
                nc.vector.scalar_tensor_tensor(
                    c2[:], gs_t[:, 0:1], -1.0, gs_t[:, 0:1], ALU.mult, ALU.mult
                )
                # h/s per partition from the first two grid columns
                hs = const_pool.tile([P, 1], F32, tag=f"hs{ib}")
                nc.vector.tensor_sub(hs[:], gs_t[:, 1:2], gs_t[:, 0:1])
                # rsc = 2h/s^2 (scale for R)
                rsc = const_pool.tile([P, 1], F32, tag=f"rsc{ib}")
                nc.vector.tensor_scalar(
                    rsc[:], hs[:], inv_t[:, 0:1], 2.0, ALU.mult, ALU.mult
                )
                # cc_k = Exp(-(h/s) * (g_k/s + g_{k+1}/s)) for k = 0..G-2
                ssum = const_pool.tile([P, G - 1], F32, tag=f"ssum{ib}")
                nc.vector.tensor_add(ssum[:], gs_t[:, 0 : G - 1], gs_t[:, 1:G])
                ccl = const_pool.tile([P, G - 1], F32, tag=f"cc{ib}")
                nc.vector.tensor_scalar_mul(ssum[:], ssum[:], hs[:, 0:1])
                nc.scalar.activation(ccl[:], ssum[:], AF.Exp, scale=-1.0)
                c1_t.append(c1)
                c2_t.append(c2)
                cc_t.append(ccl)
                rsc_t.append(rsc)

            # ---- phase 1: silu + RBF bases ------------------------------
            silu_t = [None] * IB
            bases_t = [None] * (G * IB)
            for ib in range(IB):
                x_t = x_pool.tile([P, BS], F32)
                nc.sync.dma_start(x_t[:], xT[ib * P : (ib + 1) * P, :])
                s_t = silu_pool.tile([P, BS], BF16, tag=f"silu{ib}")
                nc.scalar.activation(s_t[:], x_t[:], AF.Silu)
                silu_t[ib] = s_t
                s2_t = s2_pool.tile([P, BS], F32)
                nc.scalar.activation(s2_t[:], x_t[:], AF.Square, scale=inv_t[:, 0:1])
                r_t = r_pool.tile([P, BS], BF16)
                nc.scalar.activation(r_t[:], x_t[:], AF.Exp, scale=rsc_t[ib][:, 0:1])
                v_t = v_pool.tile([P, BS], F32)
                nc.vector.scalar_tensor_tensor(
                    v_t[:], x_t[:], c1_t[ib][:, 0:1], s2_t[:],
                    ALU.mult, ALU.subtract,
                )
                b0 = bases_pool.tile([P, BS], BF16, tag=f"bases{ib * G}")
                nc.scalar.activation(b0[:], v_t[:], AF.Exp, bias=c2_t[ib][:, 0:1])
                bases_t[ib * G] = b0
                for k in range(G - 1):
                    kc = ib * G + k
                    bt = bases_pool.tile([P, BS], BF16, tag=f"bases{kc + 1}")
                    nc.vector.scalar_tensor_tensor(
                        bt[:], r_t[:], cc_t[ib][:, k : k + 1], bases_t[kc][:],
                        ALU.mult, ALU.mult,
                    )
                    bases_t[kc + 1] = bt

            # ---- phase 2: matmuls ---------------------------------------
            # k-chunk order kc = ib*G + g matches phase-1 production order,
            # so the PE never waits long for a base tile.
            for ob in range(OB):
                o0 = ob * NO
                psums = [
                    psum_pool.tile([P, NO], F32, tag=f"ps{b}", name=f"ps{ob}_{b}")
                    for b in range(NB)
                ]

                for kc in range(KC):
                    ib, g = kc // G, kc % G
                    if g == 0:
                        # base-matmul chunk + fresh scale_spline chunk per ib
                        sb_f = sbf_pool.tile([P, NO], F32)
                        nc.sync.dma_start(
                            sb_f[:], sb[ib * P : (ib + 1) * P, o0 : o0 + NO]
                        )
                        sb_b = sbb_pool.tile([P, NO], BF16)
                        nc.vector.tensor_copy(sb_b[:], sb_f[:])
                        for b in range(NB):
                            nc.tensor.matmul(
                                psums[b][:],
                                silu_t[ib][:, b * P : (b + 1) * P],
                                sb_b[:],
                                start=(ib == 0),
                                stop=False,
                            )
                        ss_c = ss_pool.tile([P, NO], F32)
                        nc.sync.dma_start(
                            ss_c[:], ss[ib * P : (ib + 1) * P, o0 : o0 + NO]
                        )
                    w_t = w_pool.tile([P, NO], F32)
                    nc.sync.dma_start(w_t[:], wT[kc * P : (kc + 1) * P, o0 : o0 + NO])
                    w2_t = w2_pool.tile([P, NO], BF16)
                    nc.gpsimd.tensor_mul(w2_t[:], w_t[:], ss_c[:])
                    last = kc == KC - 1
                    for b in range(NB):
                        nc.tensor.matmul(
                            psums[b][:],
                            bases_t[kc][:, b * P : (b + 1) * P],
                            w2_t[:],
                            start=False,
                            stop=last,
                        )

                # drain PSUM -> SBUF -> DRAM (ACT engine; DVE stays on bases)
                for b in range(NB):
                    o_t = out_pool.tile([P, NO], F32)
                    nc.scalar.activation(o_t[:], psums[b][:], AF.Copy)
                    nc.sync.dma_start(
                        out[b * P : (b + 1) * P, o0 : o0 + NO], o_t[:]
                    )

    return nc


# ---------------------------------------------------------------------------
# host-side runner: build + compile once, then execute on 8 cores via PJRT
# ---------------------------------------------------------------------------
_STATE = {}


def _get_runner():
    if "run" in _STATE:
        return _STATE["run"]

    import jax
    from jax.sharding import Mesh, PartitionSpec
    from jax.experimental.shard_map import shard_map
    from concourse import bass2jax
    from concourse import mybir as _mb

    nc = build_bass()
    install_wait_split_hook()
    bass2jax.install_neuronx_cc_hook()

    partition_name = nc.partition_id_tensor.name if nc.partition_id_tensor else None
    in_names, out_names, out_avals, zero_shapes = [], [], [], []
    for alloc in nc.m.functions[0].allocations:
        if not isinstance(alloc, _mb.MemoryLocationSet):
            continue
        name = alloc.memorylocations[0].name
        if alloc.kind == "ExternalInput":
            if name != partition_name:
                in_names.append(name)
        elif alloc.kind == "ExternalOutput":
            out_names.append(name)
            shape = tuple(alloc.tensor_shape)
            dtype = _mb.dt.np(alloc.dtype)
            out_avals.append(jax.core.ShapedArray(shape, dtype))
            zero_shapes.append((shape, dtype))
    n_params = len(in_names)
    n_outs = len(out_avals)
    all_in_names = in_names + out_names
    if partition_name is not None:
        all_in_names = all_in_names + [partition_name]

    donate = tuple(range(n_params, n_params + n_outs))

    def _body(*args):
        operands = list(args)
        if partition_name is not None:
            operands.append(bass2jax.partition_id_tensor())
        outs = bass2jax._bass_exec_p.bind(
            *operands,
            out_avals=tuple(out_avals),
            in_names=tuple(all_in_names),
            out_names=tuple(out_names),
            lowering_input_output_aliases=(),
            sim_require_finite=True,
            sim_require_nnan=True,
            nc=nc,
        )
        return tuple(outs)

    devices = jax.devices()[:N_CORES]
    mesh = Mesh(np.asarray(devices), ("core",))
    specs = (PartitionSpec("core"),) * (n_params + n_outs)
    sharded = jax.jit(
        shard_map(
            _body,
            mesh=mesh,
            in_specs=specs,
            out_specs=(PartitionSpec("core"),) * n_outs,
            check_rep=False,
        ),
        donate_argnums=donate,
        keep_unused=True,
    )

    def run(in_maps):
        concat_in = [
            np.concatenate([np.asarray(in_maps[c][nm]) for c in range(N_CORES)], axis=0)
            for nm in in_names
        ]
        concat_zeros = [
            np.zeros((N_CORES * s[0], *s[1:]), d) for (s, d) in zero_shapes
        ]
        out_arrs = sharded(*concat_in, *concat_zeros)
        return [
            {
                nm: np.asarray(out_arrs[i]).reshape(N_CORES, *out_avals[i].shape)[c]
                for i, nm in enumerate(out_names)
            }
            for c in range(N_CORES)
        ]

    from jax.sharding import NamedSharding

    sh = NamedSharding(mesh, PartitionSpec("core"))

    def prep(in_maps):
        concat_in = [
            np.concatenate([np.asarray(in_maps[c][nm]) for c in range(N_CORES)], axis=0)
            for nm in in_names
        ]
        dev_in = [jax.device_put(a, sh) for a in concat_in]
        jax.block_until_ready(dev_in)
        return dev_in

    def exec_once(dev_in):
        zeros = [
            jax.device_put(np.zeros((N_CORES * s[0], *s[1:]), d), sh)
            for (s, d) in zero_shapes
        ]
        jax.block_until_ready(zeros)
        t0 = time.perf_counter()
        outs = sharded(*dev_in, *zeros)
        jax.block_until_ready(outs)
        return time.perf_counter() - t0

    def timed(in_maps, iters=20):
        """Steady-state timing: inputs device-resident; only fresh donated
        zero output buffers are re-staged (outside the timed region)."""
        dev_in = prep(in_maps)
        times = [exec_once(dev_in) for _ in range(iters)]
        return min(times) * 1e9, times

    _STATE["run"] = run
    _STATE["timed"] = timed
    _STATE["prep"] = prep
    _STATE["exec"] = exec_once
    _STATE["nc"] = nc
    return run


def _make_in_maps(x, scale_base, spline_weight, scale_spline, grid, sigma):
    x = np.asarray(x, np.float32)
    scale_base = np.ascontiguousarray(np.asarray(scale_base, np.float32))
    scale_spline = np.ascontiguousarray(np.asarray(scale_spline, np.float32))
    grid = np.ascontiguousarray(np.asarray(grid, np.float32))
    sigma_b = np.full((P, 1), np.float32(np.asarray(sigma)), np.float32)

    xT = np.ascontiguousarray(x.T)  # [I, B]
    # k-chunk order kc = ib*G + g: rows [kc*128,(kc+1)*128) hold (i in ib-block,
    # fixed g) — matches the bases production order in phase 1.
    wT = np.ascontiguousarray(
        np.asarray(spline_weight, np.float32)
        .transpose(1, 2, 0)          # [I, G, O]
        .reshape(I // P, P, G, O)
        .transpose(0, 2, 1, 3)       # [IB, G, P, O]
        .reshape(G * I, O)
    )

    in_maps = []
    for c in range(N_CORES):
        in_maps.append(
            {
                "xT": np.ascontiguousarray(xT[:, c * BS : (c + 1) * BS]),
                "wT": wT,
                "sb": scale_base,
                "ss": scale_spline,
                "grid": grid,
                "sigma": sigma_b,
            }
        )
    return in_maps


def kernel(x, scale_base, spline_weight, scale_spline, grid, sigma):
    run = _get_runner()
    in_maps = _make_in_maps(x, scale_base, spline_weight, scale_spline, grid, sigma)
    results = run(in_maps)
    return np.concatenate([results[c]["out"] for c in range(N_CORES)], axis=0)


def timed_run(inputs, iters=20):
    """Min wall-clock (ns) of a steady-state device-resident invocation."""
    _get_runner()
    in_maps = _make_in_maps(**inputs)
    best_ns, times = _STATE["timed"](in_maps, iters)
    ms = ", ".join(f"{t * 1e3:.2f}" for t in sorted(times)[:5])
    print(f"  fastest runs (ms): {ms}")
    return best_ns


def profile_run(inputs, outdir):
    """Capture an NTFF profile of one execution (core 0) via the axon
    sidechannel; returns (exec_time_ns, perfetto_trace_path)."""
    import glob
    import os

    from trn_agent_boot.trn_boot import _ntff_profile_via_ctypes

    import gauge.profiler
    from concourse.bass_utils import FishPath

    _get_runner()
    in_maps = _make_in_maps(**inputs)
    dev_in = _STATE["prep"](in_maps)
    _STATE["exec"](dev_in)  # warmup

    os.makedirs(outdir, exist_ok=True)
    hook = _ntff_profile_via_ctypes("/opt/axon/libaxon_pjrt.so")
    with hook(outdir, [0]):
        _STATE["exec"](dev_in)

    ntffs = glob.glob(os.path.join(outdir, "*_body*.ntff")) or glob.glob(
        os.path.join(outdir, "*.ntff")
    )
    if not ntffs:
        raise RuntimeError(f"no NTFF files written to {outdir}")
    profile = gauge.profiler.Profile(
        profile_path=FishPath(outdir),
        kernel_dev_mode=True,
        profile_on_exit=False,
        bass_kernel=_STATE["nc"].m,
        offline_processing=True,
        fname="*_body*",
    )
    results = profile.to_perfetto(model_index=(0,))
    r = results[0]
    return r.exec_time_ns, r.trace_path

